# revision 30
# baseline (speedup 1.0000x reference)
"""Multi-head causal attention on 8 TRN2 NeuronCores.

B=2, S=2048, D=1024, H=16 heads, head_dim=64. Tensor-parallel over heads:
core c owns heads {2c, 2c+1}. Each core:
  stage 1 (per 512-token piece): qT/kT/vT = W_c @ x.T (feature-major,
           bf16 matmuls, fp32 psum), then v -> token-major via PE
           transpose with a ones column per head appended (gives the
           softmax denominator for free).
  stage 2: attention in scoresT (k-major) layout, so the softmax needs
           no partition-axis reductions: exp on ScalarE (no max
           subtraction; |scores|/8 is bounded ~4), causal masking via
           precomputed triangular masks multiplied into pT, ctxT'
           accumulation = [v|1].T @ pT, normalization by the ones-row
           sum via reciprocal_approx_fast.
           Work is organized as 16 independent tasks (batch, 512-wide
           q-piece, head) executed as a 3-wide round-robin pipeline:
           while one task waits on its score matmuls, the other two
           keep ScalarE busy, which is the pacing engine.
  stage 3: partial output projection outT_c = Wo_c.T-slice @ ctxT,
           woven into the round stream one piece behind.
Host sums the 8 partial outputs and adds the bias.
"""
import numpy as np
import ml_dtypes

B, S, D, H = 2, 2048, 1024, 16
HD = 64          # head dim
NT = B * S       # 4096 tokens
P = 128          # partitions
NCORES = 8
HPC = 2          # heads per core
WIDTH = 2        # concurrent stage-2 tasks

_cache = {}


def _build():
    import concourse.bass as bass
    import concourse.mybir as mybir
    from concourse import bacc
    import concourse.tile as tile
    from concourse.masks import make_identity

    BF16 = mybir.dt.bfloat16
    F32 = mybir.dt.float32
    Exp = mybir.ActivationFunctionType.Exp

    nc = bacc.Bacc("TRN2", target_bir_lowering=False, debug=False,
                   num_devices=NCORES)

    xT_d = nc.dram_tensor("xT", [D, NT], BF16, kind="ExternalInput")
    wq_d = nc.dram_tensor("wq", [P, D], BF16, kind="ExternalInput")
    wk_d = nc.dram_tensor("wk", [P, D], BF16, kind="ExternalInput")
    wv_d = nc.dram_tensor("wv", [P, D], BF16, kind="ExternalInput")
    wo_d = nc.dram_tensor("wo", [P, D], BF16, kind="ExternalInput")
    mask_d = nc.dram_tensor("mask", [P, 4 * 512], BF16, kind="ExternalInput")
    out_d = nc.dram_tensor("out", [D, NT], BF16, kind="ExternalOutput")

    NB = S // P          # 16 k-blocks per batch
    NM = S // 512        # 4 q-pieces per batch
    NP = NT // 512       # 8 token pieces overall
    VCB = 2 * (HD + 1)   # 130: v block cols: h0 feats+1, h1 feats+1

    with tile.TileContext(nc) as tc:
        with tc.tile_pool(name="const", bufs=1) as const, \
             tc.tile_pool(name="xp", bufs=1) as xp, \
             tc.tile_pool(name="qk", bufs=1) as qk, \
             tc.tile_pool(name="misc", bufs=4) as misc, \
             tc.tile_pool(name="stg", bufs=6) as stg, \
             tc.tile_pool(name="pt", bufs=12) as ptp, \
             tc.tile_pool(name="pp", bufs=2, space="PSUM") as pp, \
             tc.tile_pool(name="sc", bufs=2, space="PSUM") as scp, \
             tc.tile_pool(name="cx", bufs=2, space="PSUM") as cxp:

            # ---- constants / weights ----
            w_sb = {}
            for name, dd in (("wq", wq_d), ("wk", wk_d), ("wv", wv_d),
                             ("wo", wo_d)):
                t = const.tile([P, D], BF16, tag=name)
                nc.sync.dma_start(t[:], dd.ap())
                w_sb[name] = t
            mask_sb = const.tile([P, 4 * 512], BF16, tag="mask")
            nc.sync.dma_start(mask_sb[:], mask_d.ap())
            ident = const.tile([P, P], BF16, tag="ident")
            make_identity(nc, ident[:])

            x_sb = [xp.tile([P, NT], BF16, tag=f"x{c}", name=f"x{c}")
                    for c in range(8)]
            qT = qk.tile([P, NT], BF16, tag="qT")
            kT = qk.tile([P, NT], BF16, tag="kT")
            vT = qk.tile([P, NT], BF16, tag="vT")
            v_sb = qk.tile([P, (NT // P) * VCB], BF16, tag="v")
            nc.gpsimd.memset(v_sb[:], 1.0)
            ctxT = qk.tile([P, NT], BF16, tag="ctxT")
            wo = w_sb["wo"]
            # pre-zero the pt ring slots so diagonal-trimmed exp ops
            # leave only zeros (not garbage) in the masked region
            for i in range(12):
                pt0 = ptp.tile([P, 1024], BF16, tag="pt", name=f"ptz{i}")
                nc.gpsimd.memset(pt0[:], 0.0)

            # hoist all x loads: the sync queue issues them before any
            # stage-3 out DMA can block it; stage-1 matmuls gate on the
            # per-piece DMA semaphores as the data lands
            for n in range(NP):
                cols = slice(n * 512, (n + 1) * 512)
                for c in range(8):
                    nc.sync.dma_start(x_sb[c][:, cols],
                                      xT_d.ap()[c * P:(c + 1) * P, cols])

            # ---- stage 1 sub-units (half-size so they slot between
            # attention score groups without starving ScalarE) ----
            s1_ps = {}

            def s1_proj_half(n, wname, dst, half):
                cols = slice(n * 512, (n + 1) * 512)
                w = w_sb[wname]
                if half == 0:
                    s1_ps[(wname, n)] = pp.tile([P, 512], F32, tag="p1",
                                                name=f"p1_{wname}_{n}")
                ps = s1_ps[(wname, n)]
                for cc in range(half * 4, half * 4 + 4):
                    nc.tensor.matmul(ps[:], w[:, cc * P:(cc + 1) * P],
                                     x_sb[cc][:, cols],
                                     start=(cc == 0), stop=(cc == 7))
                if half == 1:
                    nc.vector.tensor_copy(dst[:, cols], ps[:])
                    del s1_ps[(wname, n)]

            def s1_vtrans(n, half):
                # v -> token-major for 2 of the 4 blocks of this piece
                for t in range(4 * n + 2 * half, 4 * n + 2 * half + 2):
                    pst = pp.tile([P, P], BF16, tag="p1", name=f"ptr{t}")
                    nc.tensor.transpose(pst[:], vT[:, t * P:(t + 1) * P],
                                        ident[:])
                    # one 3D-AP copy places both heads' 64 feat cols
                    # (strides: head 65, feat 1), skipping the ones cols
                    dst3 = v_sb[:, t * VCB:(t + 1) * VCB].rearrange(
                        "p (h f) -> p h f", f=HD + 1)[:, :, 0:HD]
                    src3 = pst[:, :].rearrange("p (h f) -> p h f", f=HD)
                    nc.vector.tensor_copy(dst3, src3)

            q1 = []
            for n in range(NP):
                for wname, dst in (("wq", qT), ("wk", kT), ("wv", vT)):
                    for half in range(2):
                        q1.append((n, lambda n=n, w=wname, d=dst, h=half:
                                   s1_proj_half(n, w, d, h)))
                q1 += [(n, lambda n=n: s1_vtrans(n, 0)),
                       (n, lambda n=n: s1_vtrans(n, 1))]

            # ---- stage 2 task machinery (one task = one b, m, head) ----
            class Task:
                def __init__(self, b, m, hl):
                    self.b, self.m, self.hl = b, m, hl
                    self.jg = 0
                    self.njs = 4 * m + 4
                    self.cx = cxp.tile([HD + 1, 512], F32, tag="cx",
                                       name=f"cx_{b}_{m}_{hl}")
                    self.scs = None

                def req(self):  # stage-1 piece its next group needs
                    return self.b * NM + max(self.m, (self.jg + 1) // 4)

                def scores_alloc(self):
                    b, m, hl, jg = self.b, self.m, self.hl, self.jg
                    self.scs = scp.tile([P, 1024], F32, tag="sc",
                                        name=f"sc_{b}_{m}_{jg}_{hl}")

                def scores_mm(self, t2):
                    b, m, hl, jg = self.b, self.m, self.hl, self.jg
                    hbase = hl * HD
                    qc0 = b * S + m * 512
                    kc0 = b * S + (jg + t2) * P
                    nc.tensor.matmul(
                        self.scs[:, t2 * 512:(t2 + 1) * 512],
                        kT[hbase:hbase + HD, kc0:kc0 + P],
                        qT[hbase:hbase + HD, qc0:qc0 + 512],
                        start=True, stop=True,
                        tile_position=(hbase, 0))

                def consume(self):
                    b, m, hl, jg = self.b, self.m, self.hl, self.jg
                    pt = ptp.tile([P, 1024], BF16, tag="pt",
                                  name=f"pt_{b}_{m}_{jg}_{hl}")
                    t = jg - 4 * m
                    if t >= 2:
                        # deep-diagonal pair: cols < 128t are fully
                        # masked; skip them in the exp (mask zeroes the
                        # stale -- previously exp'd, finite -- region)
                        off = 128 * t
                        nc.scalar.activation(
                            pt[:].rearrange("p (a c) -> p a c",
                                            a=2)[:, :, off:512],
                            self.scs[:].rearrange("p (a c) -> p a c",
                                                  a=2)[:, :, off:512],
                            Exp, scale=0.125)
                    else:
                        nc.scalar.activation(pt[:], self.scs[:], Exp,
                                             scale=0.125)
                    if t >= 0:  # both js diagonal: one mask op
                        nc.vector.tensor_mul(
                            pt[:], pt[:],
                            mask_sb[:, t * 512:(t + 2) * 512])
                    for t2 in range(2):
                        j = jg + t2
                        vb = (b * NB + j) * VCB + hl * 65
                        nc.tensor.matmul(
                            self.cx[:],
                            v_sb[:, vb:vb + HD + 1],
                            pt[:, t2 * 512:(t2 + 1) * 512],
                            start=(j == 0), stop=(j == self.njs - 1))
                    self.jg += 2
                    return self.jg >= self.njs

                def normalize(self):
                    b, m, hl = self.b, self.m, self.hl
                    hbase = hl * HD
                    qc0 = b * S + m * 512
                    sm = misc.tile([1, 512], F32, tag="sm",
                                   name=f"sm_{b}_{m}_{hl}")
                    nc.vector.tensor_copy(sm[:], self.cx[HD:HD + 1, :])
                    rc = misc.tile([1, 512], F32, tag="rc",
                                   name=f"rc_{b}_{m}_{hl}")
                    nc.vector.reciprocal_approx_fast(rc[:], sm[:])
                    bc = misc.tile([HD, 512], F32, tag="bc",
                                   name=f"bc_{b}_{m}_{hl}")
                    nc.gpsimd.partition_broadcast(bc[:], rc[:])
                    nc.vector.tensor_mul(
                        ctxT[hbase:hbase + HD, qc0:qc0 + 512],
                        self.cx[0:HD, :], bc[:])

            def s3_quarter(n, qtr):
                cols = slice(n * 512, (n + 1) * 512)
                for f in range(qtr * 2, qtr * 2 + 2):
                    pso = pp.tile([P, 512], F32, tag="p1",
                                  name=f"p3_{f}_{n}")
                    nc.tensor.matmul(pso[:], wo[:, f * P:(f + 1) * P],
                                     ctxT[:, cols], start=True, stop=True)
                    st = stg.tile([P, 512], BF16, tag="st",
                                  name=f"st_{f}_{n}")
                    if f % 4 == 3:
                        nc.scalar.copy(st[:], pso[:])
                    else:
                        nc.vector.tensor_copy(st[:], pso[:])
                    nc.sync.dma_start(
                        out_d.ap()[f * P:(f + 1) * P, cols], st[:])

            # ---- the round-robin pipeline ----
            order = [(b, m, hl) for b, m in
                     [(0, 0), (0, 1), (0, 2), (0, 3),
                      (1, 1), (1, 2), (1, 3), (1, 0)]
                     for hl in range(HPC)]
            i1 = 0
            done1 = -1

            def pump_q1(need):
                nonlocal i1, done1
                while done1 < need and i1 < len(q1):
                    n, fn = q1[i1]
                    fn()
                    if i1 + 1 >= len(q1) or q1[i1 + 1][0] != n:
                        done1 = n
                    i1 += 1

            active = []
            ti = 0
            heads_done = {}
            s3q = []          # stage-3 quarters ready to emit
            round_idx = 0
            while active or ti < len(order):
                while len(active) < WIDTH and ti < len(order):
                    b, m, hl = order[ti]
                    pump_q1(b * NM + m)
                    active.append(Task(b, m, hl))
                    ti += 1
                # phase A: scores for every active task
                for t in active:
                    pump_q1(t.req())
                    t.scores_alloc()
                    t.scores_mm(0)
                    t.scores_mm(1)
                # stage-1 / stage-3 filler between the score and
                # consume phases keeps the PE queue from running dry
                if round_idx % 2 == 0 and i1 < len(q1):
                    pump_q1(q1[i1][0])
                if s3q:
                    s3_quarter(*s3q.pop(0))
                # phase B: exp/mask/ctx; retire finished tasks
                for t in list(active):
                    if t.consume():
                        t.normalize()
                        active.remove(t)
                        key = (t.b, t.m)
                        heads_done[key] = heads_done.get(key, 0) + 1
                        if heads_done[key] == HPC:
                            n = t.b * NM + t.m
                            s3q += [(n, 0), (n, 1), (n, 2), (n, 3)]
                round_idx += 1
            pump_q1(NP)
            for n, qtr in s3q:
                s3_quarter(n, qtr)
    nc.compile()
    return nc


def _get_nc():
    if "nc" not in _cache:
        _cache["nc"] = _build()
    return _cache["nc"]


def _bf16(a):
    return np.ascontiguousarray(a).astype(ml_dtypes.bfloat16)


def _prepare_in_maps(x, Wq, Wk, Wv, Wo):
    xT = _bf16(np.asarray(x, np.float32).reshape(NT, D).T)
    mask = np.zeros((P, 4 * 512), np.float32)
    pp = np.arange(P)[:, None]
    for t in range(4):
        cc = np.arange(512)[None, :]
        mask[:, t * 512:(t + 1) * 512] = (pp <= cc - 128 * t)
    mask = _bf16(mask)

    def wlayout(Wslice):  # [128 feats, 1024 d] -> [p, cc*128+f]
        return _bf16(Wslice.reshape(P, 8, P).transpose(2, 1, 0)
                     .reshape(P, D))

    in_maps = []
    for c in range(NCORES):
        rows = slice(c * P, (c + 1) * P)
        in_maps.append({
            "xT": xT,
            "wq": wlayout(np.asarray(Wq, np.float32)[rows, :]),
            "wk": wlayout(np.asarray(Wk, np.float32)[rows, :]),
            "wv": wlayout(np.asarray(Wv, np.float32)[rows, :]),
            "wo": _bf16(np.asarray(Wo, np.float32)[:, rows].T),
            "mask": mask,
        })
    return in_maps


def _run(inputs, trace=False, tmpdir=None):
    from concourse.bass_utils import run_bass_kernel_spmd
    nc = _get_nc()
    in_maps = _prepare_in_maps(inputs["x"], inputs["Wq"], inputs["Wk"],
                               inputs["Wv"], inputs["Wo"])
    res = run_bass_kernel_spmd(nc, in_maps, core_ids=list(range(NCORES)),
                               trace=trace, tmpdir=tmpdir)
    acc = np.zeros((D, NT), np.float32)
    for r in res.results:
        acc += r["out"].astype(np.float32)
    out = acc.T.reshape(B, S, D) + np.asarray(inputs["bo"], np.float32)
    return out.astype(np.float32), res


def kernel(**inputs):
    out, _ = _run(inputs)
    return out


def kernel_traced(tmpdir=None, **inputs):
    out, res = _run(inputs, trace=True, tmpdir=tmpdir)
    return out, res


# revision 31
# speedup vs baseline: 1.2157x; 1.2157x over previous
"""Multi-head causal attention on 8 TRN2 NeuronCores.

B=2, S=2048, D=1024, H=16 heads, head_dim=64. Tensor-parallel over heads:
core c owns heads {2c, 2c+1}. Each core:
  stage 1 (per 512-token piece): qT/kT/vT = W_c @ x.T (feature-major,
           bf16 matmuls, fp32 psum), then v -> token-major via PE
           transpose with a ones column per head appended (gives the
           softmax denominator for free).
  stage 2: attention in scoresT (k-major) layout, so the softmax needs
           no partition-axis reductions: exp on ScalarE (no max
           subtraction; |scores|/8 is bounded ~4), causal masking via
           precomputed triangular masks multiplied into pT, ctxT'
           accumulation = [v|1].T @ pT, normalization by the ones-row
           sum via reciprocal_approx_fast.
           Work is organized as 16 independent tasks (batch, 512-wide
           q-piece, head) executed as a 3-wide round-robin pipeline:
           while one task waits on its score matmuls, the other two
           keep ScalarE busy, which is the pacing engine.
  stage 3: partial output projection outT_c = Wo_c.T-slice @ ctxT,
           woven into the round stream one piece behind.
Host sums the 8 partial outputs and adds the bias.
"""
import numpy as np
import ml_dtypes

B, S, D, H = 2, 2048, 1024, 16
HD = 64          # head dim
NT = B * S       # 4096 tokens
P = 128          # partitions
NCORES = 8
HPC = 2          # heads per core
WIDTH = 2        # concurrent stage-2 tasks

_cache = {}


def _build():
    import concourse.bass as bass
    import concourse.mybir as mybir
    from concourse import bacc
    import concourse.tile as tile
    from concourse.masks import make_identity

    BF16 = mybir.dt.bfloat16
    F32 = mybir.dt.float32
    Exp = mybir.ActivationFunctionType.Exp

    nc = bacc.Bacc("TRN2", target_bir_lowering=False, debug=False,
                   num_devices=NCORES)

    xT_d = nc.dram_tensor("xT", [D, NT], BF16, kind="ExternalInput")
    wq_d = nc.dram_tensor("wq", [P, D], BF16, kind="ExternalInput")
    wk_d = nc.dram_tensor("wk", [P, D], BF16, kind="ExternalInput")
    wv_d = nc.dram_tensor("wv", [P, D], BF16, kind="ExternalInput")
    wo_d = nc.dram_tensor("wo", [P, D], BF16, kind="ExternalInput")
    mask_d = nc.dram_tensor("mask", [P, 4 * 512], BF16, kind="ExternalInput")
    out_d = nc.dram_tensor("out", [D, NT], BF16, kind="ExternalOutput")

    NB = S // P          # 16 k-blocks per batch
    NM = S // 512        # 4 q-pieces per batch
    NP = NT // 512       # 8 token pieces overall
    VCB = 2 * (HD + 1)   # 130: v block cols: h0 feats+1, h1 feats+1

    with tile.TileContext(nc) as tc:
        with tc.tile_pool(name="const", bufs=1) as const, \
             tc.tile_pool(name="xp", bufs=1) as xp, \
             tc.tile_pool(name="qk", bufs=1) as qk, \
             tc.tile_pool(name="misc", bufs=4) as misc, \
             tc.tile_pool(name="stg", bufs=6) as stg, \
             tc.tile_pool(name="pt", bufs=8) as ptp, \
             tc.tile_pool(name="pp", bufs=2, space="PSUM") as pp, \
             tc.tile_pool(name="sc", bufs=2, space="PSUM") as scp, \
             tc.tile_pool(name="cx", bufs=2, space="PSUM") as cxp:

            # ---- constants / weights ----
            w_sb = {}
            for name, dd in (("wq", wq_d), ("wk", wk_d), ("wv", wv_d),
                             ("wo", wo_d)):
                t = const.tile([P, D], BF16, tag=name)
                nc.sync.dma_start(t[:], dd.ap())
                w_sb[name] = t
            mask_sb = const.tile([P, 4 * 512], BF16, tag="mask")
            nc.sync.dma_start(mask_sb[:], mask_d.ap())
            ident = const.tile([P, P], BF16, tag="ident")
            make_identity(nc, ident[:])

            x_sb = [xp.tile([P, NT], BF16, tag=f"x{c}", name=f"x{c}")
                    for c in range(8)]
            qT = qk.tile([P, NT], BF16, tag="qT")
            kT = qk.tile([P, NT], BF16, tag="kT")
            vT = qk.tile([P, NT], BF16, tag="vT")
            v_sb = qk.tile([P, (NT // P) * VCB], BF16, tag="v")
            nc.gpsimd.memset(v_sb[:], 1.0)
            ctxT = qk.tile([P, NT], BF16, tag="ctxT")
            wo = w_sb["wo"]
            # pre-zero the pt ring slots so diagonal-trimmed exp ops
            # leave only zeros (not garbage) in the masked region
            for i in range(8):
                pt0 = ptp.tile([P, 1024], BF16, tag="pt", name=f"ptz{i}")
                nc.gpsimd.memset(pt0[:], 0.0)

            # hoist all x loads: the sync queue issues them before any
            # stage-3 out DMA can block it; stage-1 matmuls gate on the
            # per-piece DMA semaphores as the data lands
            for n in range(NP):
                cols = slice(n * 512, (n + 1) * 512)
                for c in range(8):
                    nc.sync.dma_start(x_sb[c][:, cols],
                                      xT_d.ap()[c * P:(c + 1) * P, cols])

            # ---- stage 1 sub-units (half-size so they slot between
            # attention score groups without starving ScalarE) ----
            s1_ps = {}

            def s1_proj_half(n, wname, dst, half):
                cols = slice(n * 512, (n + 1) * 512)
                w = w_sb[wname]
                if half == 0:
                    s1_ps[(wname, n)] = pp.tile([P, 512], F32, tag="p1",
                                                name=f"p1_{wname}_{n}")
                ps = s1_ps[(wname, n)]
                for cc in range(half * 4, half * 4 + 4):
                    nc.tensor.matmul(ps[:], w[:, cc * P:(cc + 1) * P],
                                     x_sb[cc][:, cols],
                                     start=(cc == 0), stop=(cc == 7))
                if half == 1:
                    if wname == "wv":
                        nc.vector.tensor_copy(dst[:, cols], ps[:])
                    else:
                        # ScalarE is idle early; q/k casts go there
                        nc.scalar.copy(dst[:, cols], ps[:])
                    del s1_ps[(wname, n)]

            def s1_vtrans(n, half):
                # v -> token-major for 2 of the 4 blocks of this piece
                for t in range(4 * n + 2 * half, 4 * n + 2 * half + 2):
                    pst = pp.tile([P, P], BF16, tag="p1", name=f"ptr{t}")
                    nc.tensor.transpose(pst[:], vT[:, t * P:(t + 1) * P],
                                        ident[:])
                    # one 3D-AP copy places both heads' 64 feat cols
                    # (strides: head 65, feat 1), skipping the ones cols
                    dst3 = v_sb[:, t * VCB:(t + 1) * VCB].rearrange(
                        "p (h f) -> p h f", f=HD + 1)[:, :, 0:HD]
                    src3 = pst[:, :].rearrange("p (h f) -> p h f", f=HD)
                    nc.vector.tensor_copy(dst3, src3)

            q1 = []
            for n in range(NP):
                for wname, dst in (("wq", qT), ("wk", kT), ("wv", vT)):
                    for half in range(2):
                        q1.append((n, lambda n=n, w=wname, d=dst, h=half:
                                   s1_proj_half(n, w, d, h)))
                q1 += [(n, lambda n=n: s1_vtrans(n, 0)),
                       (n, lambda n=n: s1_vtrans(n, 1))]

            # ---- stage 2 task machinery (one task = one b, m, head) ----
            class Task:
                def __init__(self, b, m, hl):
                    self.b, self.m, self.hl = b, m, hl
                    self.jg = 0
                    self.njs = 4 * m + 4
                    self.cx = cxp.tile([HD + 1, 512], F32, tag="cx",
                                       name=f"cx_{b}_{m}_{hl}")
                    self.scs = None

                def req(self):  # stage-1 piece its next group needs
                    return self.b * NM + max(self.m, (self.jg + 1) // 4)

                def scores_alloc(self):
                    b, m, hl, jg = self.b, self.m, self.hl, self.jg
                    self.scs = scp.tile([P, 1024], F32, tag="sc",
                                        name=f"sc_{b}_{m}_{jg}_{hl}")

                def scores_mm(self, t2):
                    b, m, hl, jg = self.b, self.m, self.hl, self.jg
                    hbase = hl * HD
                    qc0 = b * S + m * 512
                    kc0 = b * S + (jg + t2) * P
                    nc.tensor.matmul(
                        self.scs[:, t2 * 512:(t2 + 1) * 512],
                        kT[hbase:hbase + HD, kc0:kc0 + P],
                        qT[hbase:hbase + HD, qc0:qc0 + 512],
                        start=True, stop=True,
                        tile_position=(hbase, 0))

                def consume(self):
                    b, m, hl, jg = self.b, self.m, self.hl, self.jg
                    pt = ptp.tile([P, 1024], BF16, tag="pt",
                                  name=f"pt_{b}_{m}_{jg}_{hl}")
                    t = jg - 4 * m
                    if t >= 2:
                        # deep-diagonal pair: cols < 128t are fully
                        # masked; skip them in the exp (mask zeroes the
                        # stale -- previously exp'd, finite -- region)
                        off = 128 * t
                        nc.scalar.activation(
                            pt[:].rearrange("p (a c) -> p a c",
                                            a=2)[:, :, off:512],
                            self.scs[:].rearrange("p (a c) -> p a c",
                                                  a=2)[:, :, off:512],
                            Exp, scale=0.125)
                    else:
                        nc.scalar.activation(pt[:], self.scs[:], Exp,
                                             scale=0.125)
                    if t >= 0:  # both js diagonal: one mask op
                        nc.vector.tensor_mul(
                            pt[:], pt[:],
                            mask_sb[:, t * 512:(t + 2) * 512])
                    for t2 in range(2):
                        j = jg + t2
                        vb = (b * NB + j) * VCB + hl * 65
                        nc.tensor.matmul(
                            self.cx[:],
                            v_sb[:, vb:vb + HD + 1],
                            pt[:, t2 * 512:(t2 + 1) * 512],
                            start=(j == 0), stop=(j == self.njs - 1))
                    self.jg += 2
                    return self.jg >= self.njs

                def normalize(self):
                    b, m, hl = self.b, self.m, self.hl
                    hbase = hl * HD
                    qc0 = b * S + m * 512
                    sm = misc.tile([1, 512], F32, tag="sm",
                                   name=f"sm_{b}_{m}_{hl}")
                    nc.vector.tensor_copy(sm[:], self.cx[HD:HD + 1, :])
                    rc = misc.tile([1, 512], F32, tag="rc",
                                   name=f"rc_{b}_{m}_{hl}")
                    nc.vector.reciprocal_approx_fast(rc[:], sm[:])
                    bc = misc.tile([HD, 512], F32, tag="bc",
                                   name=f"bc_{b}_{m}_{hl}")
                    nc.gpsimd.partition_broadcast(bc[:], rc[:])
                    nc.vector.tensor_mul(
                        ctxT[hbase:hbase + HD, qc0:qc0 + 512],
                        self.cx[0:HD, :], bc[:])

            def s3_quarter(n, qtr):
                cols = slice(n * 512, (n + 1) * 512)
                for f in range(qtr * 2, qtr * 2 + 2):
                    pso = pp.tile([P, 512], F32, tag="p1",
                                  name=f"p3_{f}_{n}")
                    nc.tensor.matmul(pso[:], wo[:, f * P:(f + 1) * P],
                                     ctxT[:, cols], start=True, stop=True)
                    st = stg.tile([P, 512], BF16, tag="st",
                                  name=f"st_{f}_{n}")
                    if f % 4 == 3:
                        nc.scalar.copy(st[:], pso[:])
                    else:
                        nc.vector.tensor_copy(st[:], pso[:])
                    nc.sync.dma_start(
                        out_d.ap()[f * P:(f + 1) * P, cols], st[:])

            # ---- the round-robin pipeline ----
            order = [(b, m, hl) for b, m in
                     [(0, 0), (0, 1), (0, 2), (0, 3),
                      (1, 1), (1, 2), (1, 3), (1, 0)]
                     for hl in range(HPC)]
            i1 = 0
            done1 = -1

            def pump_q1(need):
                nonlocal i1, done1
                while done1 < need and i1 < len(q1):
                    n, fn = q1[i1]
                    fn()
                    if i1 + 1 >= len(q1) or q1[i1 + 1][0] != n:
                        done1 = n
                    i1 += 1

            active = []
            ti = 0
            heads_done = {}
            s3q = []          # stage-3 quarters ready to emit
            round_idx = 0
            while active or ti < len(order):
                while len(active) < WIDTH and ti < len(order):
                    b, m, hl = order[ti]
                    pump_q1(b * NM + m)
                    active.append(Task(b, m, hl))
                    ti += 1
                # phase A: scores for every active task
                for t in active:
                    pump_q1(t.req())
                    t.scores_alloc()
                    t.scores_mm(0)
                    t.scores_mm(1)
                # stage-1 / stage-3 filler between the score and
                # consume phases keeps the PE queue from running dry
                if round_idx % 2 == 0 and i1 < len(q1):
                    pump_q1(q1[i1][0])
                if s3q:
                    s3_quarter(*s3q.pop(0))
                # phase B: exp/mask/ctx; retire finished tasks
                for t in list(active):
                    if t.consume():
                        t.normalize()
                        active.remove(t)
                        key = (t.b, t.m)
                        heads_done[key] = heads_done.get(key, 0) + 1
                        if heads_done[key] == HPC:
                            n = t.b * NM + t.m
                            s3q += [(n, 0), (n, 1), (n, 2), (n, 3)]
                round_idx += 1
            pump_q1(NP)
            for n, qtr in s3q:
                s3_quarter(n, qtr)
    nc.compile()
    return nc


def _get_nc():
    if "nc" not in _cache:
        _cache["nc"] = _build()
    return _cache["nc"]


def _bf16(a):
    return np.ascontiguousarray(a).astype(ml_dtypes.bfloat16)


def _prepare_in_maps(x, Wq, Wk, Wv, Wo):
    xT = _bf16(np.asarray(x, np.float32).reshape(NT, D).T)
    mask = np.zeros((P, 4 * 512), np.float32)
    pp = np.arange(P)[:, None]
    for t in range(4):
        cc = np.arange(512)[None, :]
        mask[:, t * 512:(t + 1) * 512] = (pp <= cc - 128 * t)
    mask = _bf16(mask)

    def wlayout(Wslice):  # [128 feats, 1024 d] -> [p, cc*128+f]
        return _bf16(Wslice.reshape(P, 8, P).transpose(2, 1, 0)
                     .reshape(P, D))

    in_maps = []
    for c in range(NCORES):
        rows = slice(c * P, (c + 1) * P)
        in_maps.append({
            "xT": xT,
            "wq": wlayout(np.asarray(Wq, np.float32)[rows, :]),
            "wk": wlayout(np.asarray(Wk, np.float32)[rows, :]),
            "wv": wlayout(np.asarray(Wv, np.float32)[rows, :]),
            "wo": _bf16(np.asarray(Wo, np.float32)[:, rows].T),
            "mask": mask,
        })
    return in_maps


def _run(inputs, trace=False, tmpdir=None):
    from concourse.bass_utils import run_bass_kernel_spmd
    nc = _get_nc()
    in_maps = _prepare_in_maps(inputs["x"], inputs["Wq"], inputs["Wk"],
                               inputs["Wv"], inputs["Wo"])
    res = run_bass_kernel_spmd(nc, in_maps, core_ids=list(range(NCORES)),
                               trace=trace, tmpdir=tmpdir)
    acc = np.zeros((D, NT), np.float32)
    for r in res.results:
        acc += r["out"].astype(np.float32)
    out = acc.T.reshape(B, S, D) + np.asarray(inputs["bo"], np.float32)
    return out.astype(np.float32), res


def kernel(**inputs):
    out, _ = _run(inputs)
    return out


def kernel_traced(tmpdir=None, **inputs):
    out, res = _run(inputs, trace=True, tmpdir=tmpdir)
    return out, res
